# revision 24
# baseline (speedup 1.0000x reference)
"""Trainium2 Bass kernel for a 4-layer minGRU stack (log-space scan reference).

Problem shapes (hardcoded): B=8, S=2048, D=H=1024, L=4, 2H=2048.

Strategy:
  * Data-parallel over batch: 1 sample per NeuronCore (8 cores).
  * Everything on-device lives in "transposed" layout (channels on SBUF
    partitions, sequence on the free dim), so no on-device transposes are
    needed anywhere: the input x[b] is pre-transposed on the host, the
    residual stream stays transposed across layers, and the host transposes
    the final output back.
  * Per layer: GH^T = W^T.T @ X^T as float32r matmuls (full PE rate for
    moving dim >= 256), then sigmoid-only elementwise math:
        a   = sigmoid(-(gate+bg))            # 1 - z
        z   = sigmoid(gate+bg)
        g   = max(sigmoid(hid+bh), hid+bh+0.5)   # exact identity for
                                                 # g(x)=relu(x)+.5 / sigmoid(x)
        b_t = z * g
        h_t = a_t * h_{t-1} + b_t            # DVE tensor_tensor_scan
    followed by the residual add (kept in f32r for the next layer's matmul).
"""

import sys

for _p in ("/opt/trn_rl_repo", "/root/.axon_site/_ro/trn_rl_repo"):
    if _p not in sys.path:
        sys.path.insert(0, _p)

import numpy as np

import concourse.bass as bass
import concourse.mybir as mybir
import concourse.tile as tile
from concourse import bacc
from concourse.bass_utils import run_bass_kernel_spmd

# problem constants
B = 8
S = 2048
D = 1024
H = 1024
L = 4
P = 128
KT = D // P      # 8 k-tiles of the contraction dim
HT = H // P      # 8 h-tiles (output channel pairs: gate e=i, hidden e=i+8)
CH = 512         # sequence chunk (one PSUM bank of fp32)
NCH = S // CH    # 4 chunks

F32 = mybir.dt.float32
F32R = mybir.dt.float32r
ALU = mybir.AluOpType
AF = mybir.ActivationFunctionType

_cached = None


def _build():
    nc = bacc.Bacc()

    xT_d = nc.dram_tensor("xT", [D, S], F32R, kind="ExternalInput")
    # wT[l, i, p, (g, k, f)] = W_l[128*(i + 8g) + f, 128k + p]: all weights a
    # pair (gate e=i, hidden e=i+8) needs, contiguous per partition, so one
    # 1MB DMA with 8KB descriptors loads a pair
    wT_d = nc.dram_tensor("wT", [L, HT, P, 2 * KT * P], F32R, kind="ExternalInput")
    bias_d = nc.dram_tensor("bias", [L, 2 * H], F32, kind="ExternalInput")
    hin_d = nc.dram_tensor("hin", [L, H], F32, kind="ExternalInput")

    outT_d = nc.dram_tensor("outT", [D, S], F32, kind="ExternalOutput")
    fin_d = nc.dram_tensor("fin", [L, H], F32, kind="ExternalOutput")

    with tile.TileContext(nc) as tc:
        with (
            tc.tile_pool(name="resid", bufs=1) as resid,
            tc.tile_pool(name="wpool", bufs=4) as wpool,
            tc.tile_pool(name="bpool", bufs=2) as bpool,
            tc.tile_pool(name="ew", bufs=3) as ew,
            tc.tile_pool(name="hpool", bufs=3) as hpool,
            tc.tile_pool(name="psum", bufs=3, space="PSUM") as psum,
        ):
            # residual-stream ping-pong buffers, one (128, S) tile per k-tile
            bufA = []
            bufB = []
            for k in range(KT):
                tA = resid.tile([P, S], F32R, name=f"bufA{k}")
                tB = resid.tile([P, S], F32R, name=f"bufB{k}")
                bufA.append(tA)
                bufB.append(tB)
            # weight prefetch machinery: one pair ahead in steady state,
            # layer-0 pairs 0-2 interleaved with the x half-loads so the
            # PE can start as soon as w(0,0) + the first x half have landed
            wq = {}

            def load_w(l, i):
                wgh = wpool.tile([P, 2 * KT * P], F32R, name="wgh")
                nc.sync.dma_start(wgh[:], wT_d.ap()[l, i, :, :])
                wq[(l, i)] = wgh

            def load_x_range(c0, c1):
                for k in range(KT):
                    nc.sync.dma_start(
                        bufA[k][:, c0:c1],
                        xT_d.ap()[bass.ts(k, P), c0:c1],
                    )

            load_w(0, 0)
            load_x_range(0, 2 * CH)
            load_x_range(2 * CH, 4 * CH)
            load_w(0, 1)
            load_w(0, 2)

            for l in range(L):
                inbuf = bufA if l % 2 == 0 else bufB
                outbuf = bufB if l % 2 == 0 else bufA

                # per-layer bias prep: (128, HT) tiles, e-tile index on free dim
                bg = bpool.tile([P, HT], F32, name="bg")
                bh = bpool.tile([P, HT], F32, name="bh")
                nc.sync.dma_start(
                    bg[:], bias_d.ap()[l, 0:H].rearrange("(e p) -> p e", p=P)
                )
                nc.sync.dma_start(
                    bh[:], bias_d.ap()[l, H : 2 * H].rearrange("(e p) -> p e", p=P)
                )
                nbg = bpool.tile([P, HT], F32, name="nbg")
                bh5 = bpool.tile([P, HT], F32, name="bh5")
                nc.vector.tensor_scalar_mul(nbg[:], bg[:], -1.0)
                nc.vector.tensor_scalar_add(bh5[:], bh[:], 0.5)

                # initial hidden state: ginit = max(sigmoid(h0), h0 + 0.5)
                h0 = bpool.tile([P, HT], F32, name="h0")
                nc.sync.dma_start(
                    h0[:], hin_d.ap()[l, :].rearrange("(i p) -> p i", p=P)
                )
                sgh = bpool.tile([P, HT], F32, name="sgh")
                nc.scalar.activation(sgh[:], h0[:], AF.Sigmoid)
                ginit = bpool.tile([P, HT], F32, name="ginit")
                nc.vector.scalar_tensor_tensor(
                    ginit[:], h0[:], 0.5, sgh[:], op0=ALU.add, op1=ALU.max
                )

                fint = bpool.tile([P, HT], F32, name="fint")

                for i in range(HT):
                    # weights for this h-tile: gate group (e=i), hidden (e=i+HT)
                    if (l, i) not in wq:
                        load_w(l, i)
                    wgh = wq.pop((l, i))
                    nxt = (l, i + 1) if i + 1 < HT else (l + 1, 0)
                    if nxt[0] < L and nxt not in wq:
                        load_w(*nxt)

                    h_prev = None
                    for s in range(NCH):
                        ssl = bass.ts(s, CH)
                        gate_ps = psum.tile([P, CH], F32, name="gate_ps")
                        hid_ps = psum.tile([P, CH], F32, name="hid_ps")
                        for k in range(KT):
                            nc.tensor.matmul(
                                gate_ps[:],
                                wgh[:, bass.ts(k, P)],
                                inbuf[k][:, ssl],
                                start=(k == 0),
                                stop=(k == KT - 1),
                            )
                        for k in range(KT):
                            nc.tensor.matmul(
                                hid_ps[:],
                                wgh[:, bass.ds(KT * P + k * P, P)],
                                inbuf[k][:, ssl],
                                start=(k == 0),
                                stop=(k == KT - 1),
                            )

                        a_t = ew.tile([P, CH], F32, name="a_t")
                        z_t = ew.tile([P, CH], F32, name="z_t", bufs=2)
                        sg_t = ew.tile([P, CH], F32, name="sg_t", bufs=2)
                        g_t = ew.tile([P, CH], F32, name="g_t")
                        b_t = ew.tile([P, CH], F32, name="b_t")
                        nc.scalar.activation(
                            a_t[:], gate_ps[:], AF.Sigmoid,
                            bias=nbg[:, i : i + 1], scale=-1.0,
                        )
                        nc.scalar.activation(
                            z_t[:], gate_ps[:], AF.Sigmoid, bias=bg[:, i : i + 1]
                        )
                        nc.scalar.activation(
                            sg_t[:], hid_ps[:], AF.Sigmoid, bias=bh[:, i : i + 1]
                        )
                        nc.vector.scalar_tensor_tensor(
                            g_t[:], hid_ps[:], bh5[:, i : i + 1], sg_t[:],
                            op0=ALU.add, op1=ALU.max,
                        )
                        nc.vector.tensor_mul(b_t[:], z_t[:], g_t[:])

                        h_t = hpool.tile([P, CH], F32, name="h_t")
                        init = ginit[:, i : i + 1] if s == 0 else h_prev[:, CH - 1 : CH]
                        nc.vector.tensor_tensor_scan(
                            h_t[:], a_t[:], b_t[:], init,
                            op0=ALU.mult, op1=ALU.add,
                        )
                        h_prev = h_t

                        if s == NCH - 1:
                            nc.gpsimd.tensor_copy(
                                fint[:, i : i + 1], h_t[:, CH - 1 : CH]
                            )

                        # residual (feeds the next layer / final output);
                        # out dtype stays f32r so the next layer's fp32r
                        # matmul passes BIR verification. GPSIMD is idle,
                        # keep this off the (busy) DVE.
                        nc.gpsimd.tensor_add(
                            outbuf[i][:, ssl],
                            h_t[:],
                            inbuf[i][:, ssl].bitcast(F32),
                        )
                    if l == L - 1:
                        # stream the final output per chunk (short tail)
                        for so in range(NCH):
                            nc.sync.dma_start(
                                outT_d.ap()[bass.ts(i, P), bass.ts(so, CH)],
                                outbuf[i][:, bass.ts(so, CH)].bitcast(F32),
                            )

                nc.gpsimd.dma_start(
                    fin_d.ap()[l, :].rearrange("(i p) -> p i", p=P), fint[:]
                )

    nc.compile()
    return nc


def _get_nc():
    global _cached
    if _cached is None:
        _cached = _build()
    return _cached


def kernel(x, h, W0, b0, Wl, bl, _trace=False, _trace_kwargs=None):
    x = np.asarray(x, np.float32)
    h = np.asarray(h, np.float32)
    W0 = np.asarray(W0, np.float32)
    b0 = np.asarray(b0, np.float32)
    Wl = np.asarray(Wl, np.float32)
    bl = np.asarray(bl, np.float32)

    Ws = np.concatenate([W0[None], Wl], axis=0)          # (L, 2H, D)
    # wT[l, i, p, (g, k, f)] = W_l[128*(i + 8g) + f, 128k + p]
    wT = np.ascontiguousarray(
        Ws.transpose(0, 2, 1)                            # (L, D, 2H)
        .reshape(L, KT, P, 2, HT, P)                     # [l, k, p, g, i, f]
        .transpose(0, 4, 2, 3, 1, 5)                     # [l, i, p, g, k, f]
        .reshape(L, HT, P, 2 * KT * P)
    )
    bias = np.ascontiguousarray(np.concatenate([b0[None], bl], axis=0))  # (L, 2H)

    in_maps = []
    for b in range(B):
        in_maps.append(
            {
                "xT": np.ascontiguousarray(x[b].T),      # (D, S)
                "wT": wT,
                "bias": bias,
                "hin": np.ascontiguousarray(h[:, b, 0, :]),  # (L, H)
            }
        )

    nc = _get_nc()
    res = run_bass_kernel_spmd(
        nc, in_maps, list(range(B)),
        trace=_trace, **(_trace_kwargs or {}),
    )

    out = np.empty((B, S, H), np.float32)
    finals = np.empty((L, B, 1, H), np.float32)
    for b in range(B):
        out[b] = res.results[b]["outT"].T
        finals[:, b, 0, :] = res.results[b]["fin"]

    kernel._last_result = res
    return out, finals


# revision 25
# speedup vs baseline: 1.0288x; 1.0288x over previous
"""Trainium2 Bass kernel for a 4-layer minGRU stack (log-space scan reference).

Problem shapes (hardcoded): B=8, S=2048, D=H=1024, L=4, 2H=2048.

Strategy:
  * Data-parallel over batch: 1 sample per NeuronCore (8 cores).
  * Everything on-device lives in "transposed" layout (channels on SBUF
    partitions, sequence on the free dim), so no on-device transposes are
    needed anywhere: the input x[b] is pre-transposed on the host, the
    residual stream stays transposed across layers, and the host transposes
    the final output back.
  * Per layer: GH^T = W^T.T @ X^T as float32r matmuls (full PE rate for
    moving dim >= 256), then sigmoid-only elementwise math:
        a   = sigmoid(-(gate+bg))            # 1 - z
        z   = sigmoid(gate+bg)
        g   = max(sigmoid(hid+bh), hid+bh+0.5)   # exact identity for
                                                 # g(x)=relu(x)+.5 / sigmoid(x)
        b_t = z * g
        h_t = a_t * h_{t-1} + b_t            # DVE tensor_tensor_scan
    followed by the residual add (kept in f32r for the next layer's matmul).
"""

import sys

for _p in ("/opt/trn_rl_repo", "/root/.axon_site/_ro/trn_rl_repo"):
    if _p not in sys.path:
        sys.path.insert(0, _p)

import numpy as np

import concourse.bass as bass
import concourse.mybir as mybir
import concourse.tile as tile
from concourse import bacc
from concourse.bass_utils import run_bass_kernel_spmd

# problem constants
B = 8
S = 2048
D = 1024
H = 1024
L = 4
P = 128
KT = D // P      # 8 k-tiles of the contraction dim
HT = H // P      # 8 h-tiles (output channel pairs: gate e=i, hidden e=i+8)
CH = 512         # sequence chunk (one PSUM bank of fp32)
NCH = S // CH    # 4 chunks

F32 = mybir.dt.float32
F32R = mybir.dt.float32r
ALU = mybir.AluOpType
AF = mybir.ActivationFunctionType

_cached = None


def _build():
    nc = bacc.Bacc()

    xT_d = nc.dram_tensor("xT", [D, S], F32R, kind="ExternalInput")
    # wT[l, i, p, (g, k, f)] = W_l[128*(i + 8g) + f, 128k + p]: all weights a
    # pair (gate e=i, hidden e=i+8) needs, contiguous per partition, so one
    # 1MB DMA with 8KB descriptors loads a pair
    wT_d = nc.dram_tensor("wT", [L, HT, P, 2 * KT * P], F32R, kind="ExternalInput")
    bias_d = nc.dram_tensor("bias", [L, 2 * H], F32, kind="ExternalInput")
    hin_d = nc.dram_tensor("hin", [L, H], F32, kind="ExternalInput")

    outT_d = nc.dram_tensor("outT", [D, S], F32, kind="ExternalOutput")
    fin_d = nc.dram_tensor("fin", [L, H], F32, kind="ExternalOutput")

    with tile.TileContext(nc) as tc:
        with (
            tc.tile_pool(name="resid", bufs=1) as resid,
            tc.tile_pool(name="wpool", bufs=4) as wpool,
            tc.tile_pool(name="bpool", bufs=2) as bpool,
            tc.tile_pool(name="ew", bufs=3) as ew,
            tc.tile_pool(name="hpool", bufs=3) as hpool,
            tc.tile_pool(name="psum", bufs=3, space="PSUM") as psum,
        ):
            # residual-stream ping-pong buffers, one (128, S) tile per k-tile
            bufA = []
            bufB = []
            for k in range(KT):
                tA = resid.tile([P, S], F32R, name=f"bufA{k}")
                tB = resid.tile([P, S], F32R, name=f"bufB{k}")
                bufA.append(tA)
                bufB.append(tB)
            # weight prefetch machinery: one pair ahead in steady state,
            # layer-0 pairs 0-2 interleaved with the x half-loads so the
            # PE can start as soon as w(0,0) + the first x half have landed
            wq = {}

            def load_w(l, i):
                wgh = wpool.tile([P, 2 * KT * P], F32R, name="wgh")
                nc.sync.dma_start(wgh[:], wT_d.ap()[l, i, :, :])
                wq[(l, i)] = wgh

            def load_x_range(c0, c1):
                for k in range(KT):
                    nc.sync.dma_start(
                        bufA[k][:, c0:c1],
                        xT_d.ap()[bass.ts(k, P), c0:c1],
                    )

            load_w(0, 0)
            load_x_range(0, 2 * CH)
            load_w(0, 1)
            load_x_range(2 * CH, 4 * CH)
            load_w(0, 2)

            for l in range(L):
                inbuf = bufA if l % 2 == 0 else bufB
                outbuf = bufB if l % 2 == 0 else bufA

                # per-layer bias prep: (128, HT) tiles, e-tile index on free dim
                bg = bpool.tile([P, HT], F32, name="bg")
                bh = bpool.tile([P, HT], F32, name="bh")
                nc.sync.dma_start(
                    bg[:], bias_d.ap()[l, 0:H].rearrange("(e p) -> p e", p=P)
                )
                nc.sync.dma_start(
                    bh[:], bias_d.ap()[l, H : 2 * H].rearrange("(e p) -> p e", p=P)
                )
                nbg = bpool.tile([P, HT], F32, name="nbg")
                bh5 = bpool.tile([P, HT], F32, name="bh5")
                nc.vector.tensor_scalar_mul(nbg[:], bg[:], -1.0)
                nc.vector.tensor_scalar_add(bh5[:], bh[:], 0.5)

                # initial hidden state: ginit = max(sigmoid(h0), h0 + 0.5)
                h0 = bpool.tile([P, HT], F32, name="h0")
                nc.sync.dma_start(
                    h0[:], hin_d.ap()[l, :].rearrange("(i p) -> p i", p=P)
                )
                sgh = bpool.tile([P, HT], F32, name="sgh")
                nc.scalar.activation(sgh[:], h0[:], AF.Sigmoid)
                ginit = bpool.tile([P, HT], F32, name="ginit")
                nc.vector.scalar_tensor_tensor(
                    ginit[:], h0[:], 0.5, sgh[:], op0=ALU.add, op1=ALU.max
                )

                fint = bpool.tile([P, HT], F32, name="fint")

                for i in range(HT):
                    # weights for this h-tile: gate group (e=i), hidden (e=i+HT)
                    if (l, i) not in wq:
                        load_w(l, i)
                    wgh = wq.pop((l, i))
                    nxt = (l, i + 1) if i + 1 < HT else (l + 1, 0)
                    if nxt[0] < L and nxt not in wq:
                        load_w(*nxt)

                    h_prev = None
                    for s in range(NCH):
                        ssl = bass.ts(s, CH)
                        gate_ps = psum.tile([P, CH], F32, name="gate_ps")
                        hid_ps = psum.tile([P, CH], F32, name="hid_ps")
                        for k in range(KT):
                            nc.tensor.matmul(
                                gate_ps[:],
                                wgh[:, bass.ts(k, P)],
                                inbuf[k][:, ssl],
                                start=(k == 0),
                                stop=(k == KT - 1),
                            )
                        for k in range(KT):
                            nc.tensor.matmul(
                                hid_ps[:],
                                wgh[:, bass.ds(KT * P + k * P, P)],
                                inbuf[k][:, ssl],
                                start=(k == 0),
                                stop=(k == KT - 1),
                            )

                        a_t = ew.tile([P, CH], F32, name="a_t")
                        z_t = ew.tile([P, CH], F32, name="z_t", bufs=2)
                        sg_t = ew.tile([P, CH], F32, name="sg_t", bufs=2)
                        g_t = ew.tile([P, CH], F32, name="g_t")
                        b_t = ew.tile([P, CH], F32, name="b_t")
                        nc.scalar.activation(
                            a_t[:], gate_ps[:], AF.Sigmoid,
                            bias=nbg[:, i : i + 1], scale=-1.0,
                        )
                        nc.scalar.activation(
                            z_t[:], gate_ps[:], AF.Sigmoid, bias=bg[:, i : i + 1]
                        )
                        nc.scalar.activation(
                            sg_t[:], hid_ps[:], AF.Sigmoid, bias=bh[:, i : i + 1]
                        )
                        nc.vector.scalar_tensor_tensor(
                            g_t[:], hid_ps[:], bh5[:, i : i + 1], sg_t[:],
                            op0=ALU.add, op1=ALU.max,
                        )
                        nc.vector.tensor_mul(b_t[:], z_t[:], g_t[:])

                        h_t = hpool.tile([P, CH], F32, name="h_t")
                        init = ginit[:, i : i + 1] if s == 0 else h_prev[:, CH - 1 : CH]
                        nc.vector.tensor_tensor_scan(
                            h_t[:], a_t[:], b_t[:], init,
                            op0=ALU.mult, op1=ALU.add,
                        )
                        h_prev = h_t

                        if s == NCH - 1:
                            nc.gpsimd.tensor_copy(
                                fint[:, i : i + 1], h_t[:, CH - 1 : CH]
                            )

                        # residual (feeds the next layer / final output);
                        # out dtype stays f32r so the next layer's fp32r
                        # matmul passes BIR verification. GPSIMD is idle,
                        # keep this off the (busy) DVE.
                        nc.gpsimd.tensor_add(
                            outbuf[i][:, ssl],
                            h_t[:],
                            inbuf[i][:, ssl].bitcast(F32),
                        )
                    if l == L - 1:
                        # stream the final output per chunk (short tail)
                        for so in range(NCH):
                            nc.sync.dma_start(
                                outT_d.ap()[bass.ts(i, P), bass.ts(so, CH)],
                                outbuf[i][:, bass.ts(so, CH)].bitcast(F32),
                            )

                nc.gpsimd.dma_start(
                    fin_d.ap()[l, :].rearrange("(i p) -> p i", p=P), fint[:]
                )

    nc.compile()
    return nc


def _get_nc():
    global _cached
    if _cached is None:
        _cached = _build()
    return _cached


def kernel(x, h, W0, b0, Wl, bl, _trace=False, _trace_kwargs=None):
    x = np.asarray(x, np.float32)
    h = np.asarray(h, np.float32)
    W0 = np.asarray(W0, np.float32)
    b0 = np.asarray(b0, np.float32)
    Wl = np.asarray(Wl, np.float32)
    bl = np.asarray(bl, np.float32)

    Ws = np.concatenate([W0[None], Wl], axis=0)          # (L, 2H, D)
    # wT[l, i, p, (g, k, f)] = W_l[128*(i + 8g) + f, 128k + p]
    wT = np.ascontiguousarray(
        Ws.transpose(0, 2, 1)                            # (L, D, 2H)
        .reshape(L, KT, P, 2, HT, P)                     # [l, k, p, g, i, f]
        .transpose(0, 4, 2, 3, 1, 5)                     # [l, i, p, g, k, f]
        .reshape(L, HT, P, 2 * KT * P)
    )
    bias = np.ascontiguousarray(np.concatenate([b0[None], bl], axis=0))  # (L, 2H)

    in_maps = []
    for b in range(B):
        in_maps.append(
            {
                "xT": np.ascontiguousarray(x[b].T),      # (D, S)
                "wT": wT,
                "bias": bias,
                "hin": np.ascontiguousarray(h[:, b, 0, :]),  # (L, H)
            }
        )

    nc = _get_nc()
    res = run_bass_kernel_spmd(
        nc, in_maps, list(range(B)),
        trace=_trace, **(_trace_kwargs or {}),
    )

    out = np.empty((B, S, H), np.float32)
    finals = np.empty((L, B, 1, H), np.float32)
    for b in range(B):
        out[b] = res.results[b]["outT"].T
        finals[:, b, 0, :] = res.results[b]["fin"]

    kernel._last_result = res
    return out, finals


# revision 26
# speedup vs baseline: 1.0353x; 1.0063x over previous
"""Trainium2 Bass kernel for a 4-layer minGRU stack (log-space scan reference).

Problem shapes (hardcoded): B=8, S=2048, D=H=1024, L=4, 2H=2048.

Strategy:
  * Data-parallel over batch: 1 sample per NeuronCore (8 cores).
  * Everything on-device lives in "transposed" layout (channels on SBUF
    partitions, sequence on the free dim), so no on-device transposes are
    needed anywhere: the input x[b] is pre-transposed on the host, the
    residual stream stays transposed across layers, and the host transposes
    the final output back.
  * Per layer: GH^T = W^T.T @ X^T as float32r matmuls (full PE rate for
    moving dim >= 256), then sigmoid-only elementwise math:
        a   = sigmoid(-(gate+bg))            # 1 - z
        z   = sigmoid(gate+bg)
        g   = max(sigmoid(hid+bh), hid+bh+0.5)   # exact identity for
                                                 # g(x)=relu(x)+.5 / sigmoid(x)
        b_t = z * g
        h_t = a_t * h_{t-1} + b_t            # DVE tensor_tensor_scan
    followed by the residual add (kept in f32r for the next layer's matmul).
"""

import sys

for _p in ("/opt/trn_rl_repo", "/root/.axon_site/_ro/trn_rl_repo"):
    if _p not in sys.path:
        sys.path.insert(0, _p)

import numpy as np

import concourse.bass as bass
import concourse.mybir as mybir
import concourse.tile as tile
from concourse import bacc
from concourse.bass_utils import run_bass_kernel_spmd

# problem constants
B = 8
S = 2048
D = 1024
H = 1024
L = 4
P = 128
KT = D // P      # 8 k-tiles of the contraction dim
HT = H // P      # 8 h-tiles (output channel pairs: gate e=i, hidden e=i+8)
CH = 512         # sequence chunk (one PSUM bank of fp32)
NCH = S // CH    # 4 chunks

F32 = mybir.dt.float32
F32R = mybir.dt.float32r
ALU = mybir.AluOpType
AF = mybir.ActivationFunctionType

_cached = None


def _build():
    nc = bacc.Bacc()

    xT_d = nc.dram_tensor("xT", [D, S], F32R, kind="ExternalInput")
    # wT[l, i, p, (g, k, f)] = W_l[128*(i + 8g) + f, 128k + p]: all weights a
    # pair (gate e=i, hidden e=i+8) needs, contiguous per partition, so one
    # 1MB DMA with 8KB descriptors loads a pair
    wT_d = nc.dram_tensor("wT", [L, HT, P, 2 * KT * P], F32R, kind="ExternalInput")
    bias_d = nc.dram_tensor("bias", [L, 2 * H], F32, kind="ExternalInput")
    hin_d = nc.dram_tensor("hin", [L, H], F32, kind="ExternalInput")

    outT_d = nc.dram_tensor("outT", [D, S], F32, kind="ExternalOutput")
    fin_d = nc.dram_tensor("fin", [L, H], F32, kind="ExternalOutput")

    with tile.TileContext(nc) as tc:
        with (
            tc.tile_pool(name="resid", bufs=1) as resid,
            tc.tile_pool(name="wpool", bufs=4) as wpool,
            tc.tile_pool(name="bpool", bufs=2) as bpool,
            tc.tile_pool(name="ew", bufs=3) as ew,
            tc.tile_pool(name="hpool", bufs=3) as hpool,
            tc.tile_pool(name="psum", bufs=4, space="PSUM") as psum,
        ):
            # residual-stream ping-pong buffers, one (128, S) tile per k-tile
            bufA = []
            bufB = []
            for k in range(KT):
                tA = resid.tile([P, S], F32R, name=f"bufA{k}")
                tB = resid.tile([P, S], F32R, name=f"bufB{k}")
                bufA.append(tA)
                bufB.append(tB)
            # weight prefetch machinery: one pair ahead in steady state,
            # layer-0 pairs 0-2 interleaved with the x half-loads so the
            # PE can start as soon as w(0,0) + the first x half have landed
            wq = {}

            def load_w(l, i):
                wgh = wpool.tile([P, 2 * KT * P], F32R, name="wgh")
                nc.sync.dma_start(wgh[:], wT_d.ap()[l, i, :, :])
                wq[(l, i)] = wgh

            def load_x_range(c0, c1):
                for k in range(KT):
                    nc.sync.dma_start(
                        bufA[k][:, c0:c1],
                        xT_d.ap()[bass.ts(k, P), c0:c1],
                    )

            load_w(0, 0)
            load_x_range(0, 2 * CH)
            load_w(0, 1)
            load_x_range(2 * CH, 4 * CH)
            load_w(0, 2)

            for l in range(L):
                inbuf = bufA if l % 2 == 0 else bufB
                outbuf = bufB if l % 2 == 0 else bufA

                # per-layer bias prep: (128, HT) tiles, e-tile index on free dim
                bg = bpool.tile([P, HT], F32, name="bg")
                bh = bpool.tile([P, HT], F32, name="bh")
                nc.sync.dma_start(
                    bg[:], bias_d.ap()[l, 0:H].rearrange("(e p) -> p e", p=P)
                )
                nc.sync.dma_start(
                    bh[:], bias_d.ap()[l, H : 2 * H].rearrange("(e p) -> p e", p=P)
                )
                nbg = bpool.tile([P, HT], F32, name="nbg")
                bh5 = bpool.tile([P, HT], F32, name="bh5")
                nc.vector.tensor_scalar_mul(nbg[:], bg[:], -1.0)
                nc.vector.tensor_scalar_add(bh5[:], bh[:], 0.5)

                # initial hidden state: ginit = max(sigmoid(h0), h0 + 0.5)
                h0 = bpool.tile([P, HT], F32, name="h0")
                nc.sync.dma_start(
                    h0[:], hin_d.ap()[l, :].rearrange("(i p) -> p i", p=P)
                )
                sgh = bpool.tile([P, HT], F32, name="sgh")
                nc.scalar.activation(sgh[:], h0[:], AF.Sigmoid)
                ginit = bpool.tile([P, HT], F32, name="ginit")
                nc.vector.scalar_tensor_tensor(
                    ginit[:], h0[:], 0.5, sgh[:], op0=ALU.add, op1=ALU.max
                )

                fint = bpool.tile([P, HT], F32, name="fint")

                for i in range(HT):
                    # weights for this h-tile: gate group (e=i), hidden (e=i+HT)
                    if (l, i) not in wq:
                        load_w(l, i)
                    wgh = wq.pop((l, i))
                    nxt = (l, i + 1) if i + 1 < HT else (l + 1, 0)
                    if nxt[0] < L and nxt not in wq:
                        load_w(*nxt)

                    h_prev = None
                    for s in range(NCH):
                        ssl = bass.ts(s, CH)
                        gate_ps = psum.tile([P, CH], F32, name="gate_ps")
                        hid_ps = psum.tile([P, CH], F32, name="hid_ps")
                        for k in range(KT):
                            nc.tensor.matmul(
                                gate_ps[:],
                                wgh[:, bass.ts(k, P)],
                                inbuf[k][:, ssl],
                                start=(k == 0),
                                stop=(k == KT - 1),
                            )
                        for k in range(KT):
                            nc.tensor.matmul(
                                hid_ps[:],
                                wgh[:, bass.ds(KT * P + k * P, P)],
                                inbuf[k][:, ssl],
                                start=(k == 0),
                                stop=(k == KT - 1),
                            )

                        a_t = ew.tile([P, CH], F32, name="a_t")
                        z_t = ew.tile([P, CH], F32, name="z_t", bufs=2)
                        sg_t = ew.tile([P, CH], F32, name="sg_t", bufs=2)
                        g_t = ew.tile([P, CH], F32, name="g_t")
                        b_t = ew.tile([P, CH], F32, name="b_t")
                        nc.scalar.activation(
                            a_t[:], gate_ps[:], AF.Sigmoid,
                            bias=nbg[:, i : i + 1], scale=-1.0,
                        )
                        nc.scalar.activation(
                            z_t[:], gate_ps[:], AF.Sigmoid, bias=bg[:, i : i + 1]
                        )
                        nc.scalar.activation(
                            sg_t[:], hid_ps[:], AF.Sigmoid, bias=bh[:, i : i + 1]
                        )
                        nc.vector.scalar_tensor_tensor(
                            g_t[:], hid_ps[:], bh5[:, i : i + 1], sg_t[:],
                            op0=ALU.add, op1=ALU.max,
                        )
                        nc.vector.tensor_mul(b_t[:], z_t[:], g_t[:])

                        h_t = hpool.tile([P, CH], F32, name="h_t")
                        init = ginit[:, i : i + 1] if s == 0 else h_prev[:, CH - 1 : CH]
                        nc.vector.tensor_tensor_scan(
                            h_t[:], a_t[:], b_t[:], init,
                            op0=ALU.mult, op1=ALU.add,
                        )
                        h_prev = h_t

                        if s == NCH - 1:
                            nc.gpsimd.tensor_copy(
                                fint[:, i : i + 1], h_t[:, CH - 1 : CH]
                            )

                        # residual (feeds the next layer / final output);
                        # out dtype stays f32r so the next layer's fp32r
                        # matmul passes BIR verification. GPSIMD is idle,
                        # keep this off the (busy) DVE.
                        nc.gpsimd.tensor_add(
                            outbuf[i][:, ssl],
                            h_t[:],
                            inbuf[i][:, ssl].bitcast(F32),
                        )
                    if l == L - 1:
                        # stream the final output per chunk (short tail)
                        for so in range(NCH):
                            nc.sync.dma_start(
                                outT_d.ap()[bass.ts(i, P), bass.ts(so, CH)],
                                outbuf[i][:, bass.ts(so, CH)].bitcast(F32),
                            )

                nc.gpsimd.dma_start(
                    fin_d.ap()[l, :].rearrange("(i p) -> p i", p=P), fint[:]
                )

    nc.compile()
    return nc


def _get_nc():
    global _cached
    if _cached is None:
        _cached = _build()
    return _cached


def kernel(x, h, W0, b0, Wl, bl, _trace=False, _trace_kwargs=None):
    x = np.asarray(x, np.float32)
    h = np.asarray(h, np.float32)
    W0 = np.asarray(W0, np.float32)
    b0 = np.asarray(b0, np.float32)
    Wl = np.asarray(Wl, np.float32)
    bl = np.asarray(bl, np.float32)

    Ws = np.concatenate([W0[None], Wl], axis=0)          # (L, 2H, D)
    # wT[l, i, p, (g, k, f)] = W_l[128*(i + 8g) + f, 128k + p]
    wT = np.ascontiguousarray(
        Ws.transpose(0, 2, 1)                            # (L, D, 2H)
        .reshape(L, KT, P, 2, HT, P)                     # [l, k, p, g, i, f]
        .transpose(0, 4, 2, 3, 1, 5)                     # [l, i, p, g, k, f]
        .reshape(L, HT, P, 2 * KT * P)
    )
    bias = np.ascontiguousarray(np.concatenate([b0[None], bl], axis=0))  # (L, 2H)

    in_maps = []
    for b in range(B):
        in_maps.append(
            {
                "xT": np.ascontiguousarray(x[b].T),      # (D, S)
                "wT": wT,
                "bias": bias,
                "hin": np.ascontiguousarray(h[:, b, 0, :]),  # (L, H)
            }
        )

    nc = _get_nc()
    res = run_bass_kernel_spmd(
        nc, in_maps, list(range(B)),
        trace=_trace, **(_trace_kwargs or {}),
    )

    out = np.empty((B, S, H), np.float32)
    finals = np.empty((L, B, 1, H), np.float32)
    for b in range(B):
        out[b] = res.results[b]["outT"].T
        finals[:, b, 0, :] = res.results[b]["fin"]

    kernel._last_result = res
    return out, finals
